# revision 2
# baseline (speedup 1.0000x reference)
"""Trainium2 Bass kernel for InterpBaselineEncoder (histogram binning), v2.

Reference computation (per batch b of B=4):
  - coarsen 128x128 grid by 4x4 -> 32x32=1024 cells (grid_loc = regular
    meshgrid centers, grid_val = 4x4 mean of yc_on_grid)
  - bin U=8192 off-grid points to L1-nearest cell; scatter-mean yc_off
    values + the on-grid cell value into each cell
  - bin T=4096 target points the same way and gather the cell averages

Binning is closed-form (regular grid): clamp(round(p*inv + off), 0, 31).

Scatter: one-hot matmul, cell split as (ihj=2i+jh) x (jl=j%16); psum
ps[64, (v,jl)] accumulates 64 point tiles + 8 pseudo-point tiles (the
on-grid values, one-hots precomputed host-side).  Moving operand
w2[u, v, jl]: v in [0,8) is onehot16(jl)*y (DVE+GPSIMD split), v=8 is
onehot16(jl) itself (scalar-engine copy) giving counts.

Gather: hybrid split across engines.  Path A (3/8 of targets): build a
per-partition table tbl[p, 64*jl+ihj] = avg[cell, y=p%8] via a bf16
TensorE transpose + 16 replication matmuls (64-partition groups at PE
quadrant offsets 0/64), then one GPSIMD ap_gather (each group of 16
partitions gathers its own targets; ~28-40ns/index on the Q7s).  Path B
(5/8 of targets), concurrently on DVE+PE: one-hot over ihj, TensorE
transposes, rv = onehot^T @ avg matmuls, then a batched jl-select
(mask-mult + reduce) on DVE.  Target staging is permuted host-side
(data-independent) so both paths' outputs DMA out in device-natural
layouts; the host inverts the permutation during unsharding.

Sharding: 8 cores = 4 batches x 2 target halves (scatter duplicated per
pair, gather split).  SPMD: one Bass program, per-core input maps.
"""
import sys
import numpy as np

for _p in ("/opt/trn_rl_repo", "/opt/pypackages"):
    if _p not in sys.path:
        sys.path.insert(0, _p)

import ml_dtypes  # noqa: E402
from concourse import bass, bacc, mybir, tile  # noqa: E402
from concourse.bass_utils import run_bass_kernel_spmd  # noqa: E402

F32 = mybir.dt.float32
BF16 = mybir.dt.bfloat16
I16 = mybir.dt.int16
ALU = mybir.AluOpType

B, U, T, Y = 4, 8192, 4096, 8
GI = GJ = 32           # coarse grid 32x32
TH = T // 2            # targets per core (2048)
KT = U // 128          # 64 point tiles
NT = TH // 128         # 16 target tiles
CH = 16                # point tiles per scatter chunk
KD = 9                 # DVE tiles per chunk (rest on gpsimd)

# closed-form bin constants: centers c_k = (4k+1.5)/127, step 4/127
_C0 = 1.5 / 127.0
_INV = 127.0 / 4.0
_OFF0 = float(np.float32(-_C0 * _INV))
_MAGIC = 8388608.0  # 2^23: (z + M) - M rounds z to nearest-even integer

# f32 const block [128, 80]: iota64 | iota16
_CF_COLS = 80
# bf16 const block [128, 1888]:
#   early [0:224]: pmat32 | blps128 | identb64
#   late  [224:1888]: raps512 | P64all1024 | identb128
_CB_COLS = 1888
# packed f32 input block [128, 672]
_IN_COLS = KT + KT + KT * Y + NT + NT


def _emit_bin(nc, pool, p_ap, n, nm):
    """clamp(round_ne(p*INV+OFF0), 0, 31) -> [128, n] f32 (3 vector ops)."""
    z = pool.tile([128, n], F32, tag=f"binz{nm}")
    idx = pool.tile([128, n], F32, tag=f"bini{nm}")
    nc.vector.tensor_scalar(z[:], p_ap, _INV, _OFF0, ALU.mult, ALU.add)
    nc.vector.tensor_scalar(idx[:], z[:], _MAGIC, _MAGIC, ALU.add, ALU.subtract)
    out = pool.tile([128, n], F32, tag=f"binc{nm}")
    nc.vector.tensor_scalar(out[:], idx[:], 0.0, 31.0, ALU.max, ALU.min)
    return out


def build_nc():
    nc = bacc.Bacc("TRN2", target_bir_lowering=False, debug=False)

    constF = nc.declare_dram_parameter("constF", [128, _CF_COLS], F32,
                                       isOutput=False)
    constB = nc.declare_dram_parameter("constB", [128, _CB_COLS], BF16,
                                       isOutput=False)
    inF = nc.declare_dram_parameter("inF", [128, _IN_COLS], F32, isOutput=False)
    ycON = nc.declare_dram_parameter("ycON", [128, 1024], BF16, isOutput=False)
    # device-natural output layouts; host reassembles rows (pure unshard)
    outA_d = nc.declare_dram_parameter("outA", [128, 96], F32, isOutput=True)
    outB_d = nc.declare_dram_parameter("outB", [128, 10, Y], F32, isOutput=True)

    with tile.TileContext(nc) as tc:
        with (
            tc.tile_pool(name="const", bufs=1) as cpool,
            tc.tile_pool(name="work", bufs=1) as wpool,
            tc.tile_pool(name="psP", bufs=1, space="PSUM") as psP,
            tc.tile_pool(name="psS", bufs=1, space="PSUM") as psS,
            tc.tile_pool(name="psT", bufs=1, space="PSUM") as psT,
            tc.tile_pool(name="psB", bufs=1, space="PSUM") as psB,
            tc.tile_pool(name="psR", bufs=1, space="PSUM") as psR,
        ):
            cf = cpool.tile([128, _CF_COLS], F32, tag="cf")
            cb = cpool.tile([128, _CB_COLS], BF16, tag="cb")
            tin = wpool.tile([128, _IN_COLS], F32, tag="tin")
            t_ycon = wpool.tile([128, 1024], BF16, tag="ycon")
            # two HWDGE queues (SP + Activation); binning-critical data first,
            # yc_off split so early scatter chunks aren't gated on one queue
            nc.sync.dma_start(tin[:, 0:128], inF[:, 0:128])
            nc.scalar.dma_start(t_ycon[:], ycON[:])
            nc.sync.dma_start(cf[:], constF[:])
            nc.sync.dma_start(tin[:, 128:400], inF[:, 128:400])
            nc.scalar.dma_start(tin[:, 400:672], inF[:, 400:672])
            nc.scalar.dma_start(cb[:, 0:224], constB[:, 0:224])
            nc.scalar.dma_start(cb[:, 224:_CB_COLS], constB[:, 224:_CB_COLS])

            c_iota64 = cf[:, 0:64]
            c_iota16 = cf[:, 64:80]
            c_pmat = cb[:, 0:32]
            c_blps = cb[:, 32:160].rearrange("p (m l) -> p m l", l=16)
            c_identB = cb[0:64, 160:224]
            c_raps = cb[:, 224:736].rearrange("p (m q) -> p m q", q=64)
            c_identB128 = cb[:, 1760:1888]

            o = 0
            t_py = tin[:, o:o + KT]; o += KT
            t_px = tin[:, o:o + KT]; o += KT
            t_yoff = tin[:, o:o + KT * Y].rearrange("p (k y) -> p k y", y=Y)
            o += KT * Y
            t_xty = tin[:, o:o + NT]; o += NT
            t_xtx = tin[:, o:o + NT]; o += NT

            # ---- pooling of on-grid values (PE, bf16) ----
            pp = psP.tile([32, 1024], F32, tag="pp")
            nc.tensor.matmul(pp[:, 0:512], c_pmat, t_ycon[:, 0:512],
                             start=True, stop=True)
            nc.tensor.matmul(pp[:, 512:1024], c_pmat, t_ycon[:, 512:1024],
                             start=True, stop=True)

            # ---- off-grid binning ----
            ioff = _emit_bin(nc, wpool, t_py, KT, "o")
            joff = _emit_bin(nc, wpool, t_px, KT, "o2")
            jh = wpool.tile([128, KT], F32, tag="jh")
            jh16 = wpool.tile([128, KT], F32, tag="jh16")
            jl = wpool.tile([128, KT], F32, tag="jl")
            i2 = wpool.tile([128, KT], F32, tag="i2")
            ihj = wpool.tile([128, KT], F32, tag="ihj")
            nc.vector.tensor_scalar(jh[:], joff[:], 16.0, None, ALU.is_ge)
            nc.vector.tensor_scalar(jh16[:], jh[:], 16.0, None, ALU.mult)
            nc.vector.tensor_tensor(jl[:], joff[:], jh16[:], ALU.subtract)
            nc.vector.tensor_scalar(i2[:], ioff[:], 2.0, None, ALU.mult)
            nc.vector.tensor_tensor(ihj[:], i2[:], jh[:], ALU.add)

            # ---- one-hots + w2, chunked; scatter matmuls interleaved.
            # The gva W-pool adds (DVE, dependent on the pool matmuls) are
            # emitted after chunk 0 so they don't stall the DVE FIFO.
            ra = wpool.tile([128, KT, 64], BF16, tag="ra")
            bl = wpool.tile([128, KT, 16], BF16, tag="bl")
            w2 = wpool.tile([128, KT, 9, 16], BF16, tag="w2")
            ps = psS.tile([64, 9, 16], F32, tag="ps")
            yps = wpool.tile([128, 8, Y], F32, tag="yps")
            for c0 in range(0, KT, CH):
                sl = slice(c0, c0 + CH)
                if c0 == CH:
                    # ---- W-pool: one strided reduce over c (PSUM src) ----
                    ppv = pp[:].rearrange("p (j c y) -> p j c y", c=4, y=Y)
                    gva = wpool.tile([32, GJ, Y], F32, tag="gva")
                    nc.vector.tensor_reduce(gva[:], ppv[:].transpose([0, 1, 3, 2]),
                                            axis=mybir.AxisListType.X, op=ALU.add)
                    nc.sync.dma_start(yps[:], gva[:])
                nc.vector.tensor_tensor(
                    bl[:, sl, :],
                    c_iota16.unsqueeze(1).broadcast_to((128, CH, 16)),
                    jl[:, sl].unsqueeze(2).broadcast_to((128, CH, 16)),
                    ALU.is_equal,
                )
                nc.scalar.copy(w2[:, sl, 8, :], bl[:, sl, :])
                nc.vector.tensor_tensor(
                    ra[:, sl, :],
                    c_iota64.unsqueeze(1).broadcast_to((128, CH, 64)),
                    ihj[:, sl].unsqueeze(2).broadcast_to((128, CH, 64)),
                    ALU.is_equal,
                )
                nc.vector.tensor_tensor(
                    w2[:, sl, 0:8, :],
                    t_yoff[:, sl, :].unsqueeze(3).broadcast_to((128, CH, 8, 16)),
                    bl[:, sl, :].unsqueeze(2).broadcast_to((128, CH, 8, 16)),
                    ALU.mult,
                )
                for k in range(c0, c0 + CH):
                    nc.tensor.matmul(ps[:], ra[:, k, :], w2[:, k, :, :],
                                     start=(k == 0), stop=(k == KT - 1))
                if c0 == 2 * CH:
                    # pseudo-point tiles mid-stream so their matmuls fill PE
                    # gaps during chunk 3 instead of serializing at the end
                    w2ps = wpool.tile([128, 8, 9, 16], BF16, tag="w2ps")
                    nc.vector.tensor_tensor(
                        w2ps[:, :, 0:8, :],
                        yps[:].unsqueeze(3).broadcast_to((128, 8, 8, 16)),
                        c_blps.unsqueeze(2).broadcast_to((128, 8, 8, 16)),
                        ALU.mult,
                    )
                    nc.scalar.copy(w2ps[:, :, 8, :], c_blps)
                    for m in range(8):
                        nc.tensor.matmul(ps[:], c_raps[:, m, :],
                                         w2ps[:, m, :, :],
                                         start=False, stop=False)

            # ---- target binning + gather index (fills DVE idle) ----
            # xty|xtx are adjacent columns: bin both in one 3-op pass
            bt = _emit_bin(nc, wpool, tin[:, 640:672], 2 * NT, "t")
            it = bt[:, 0:NT]
            jt = bt[:, NT:2 * NT]
            jht = wpool.tile([128, NT], F32, tag="jht")
            jh16t = wpool.tile([128, NT], F32, tag="jh16t")
            jlt = wpool.tile([128, NT], F32, tag="jlt")
            tb = wpool.tile([128, NT], F32, tag="tb")
            ihjt = wpool.tile([128, NT], F32, tag="ihjt")
            jl64 = wpool.tile([128, NT], F32, tag="jl64")
            cidx = wpool.tile([128, NT], F32, tag="cidx")
            nc.vector.tensor_scalar(jht[:], jt[:], 16.0, None, ALU.is_ge)
            nc.vector.tensor_scalar(jh16t[:], jht[:], 16.0, None, ALU.mult)
            nc.vector.tensor_tensor(jlt[:], jt[:], jh16t[:], ALU.subtract)
            nc.vector.tensor_scalar(tb[:], it[:], 2.0, None, ALU.mult)
            nc.vector.tensor_tensor(ihjt[:], tb[:], jht[:], ALU.add)
            nc.vector.tensor_scalar(jl64[:], jlt[:], 64.0, None, ALU.mult)
            nc.vector.tensor_tensor(cidx[:], jl64[:], ihjt[:], ALU.add)
            # GP half: idx slots 0:6 (within-group targets 0..95)
            idx16 = wpool.tile([128, 6], I16, tag="idx16")
            nc.vector.tensor_copy(idx16[:], cidx[:, 0:6])
            # DVE half: one-hot over ihj + jl mask for slots 6:16
            rat = wpool.tile([128, 10, 64], BF16, tag="rat")
            nc.vector.tensor_tensor(
                rat[:],
                c_iota64.unsqueeze(1).broadcast_to((128, 10, 64)),
                ihjt[:, 6:16].unsqueeze(2).broadcast_to((128, 10, 64)),
                ALU.is_equal,
            )
            zttl2 = wpool.tile([128, 10, 16], BF16, tag="zttl2")
            nc.vector.tensor_tensor(
                zttl2[:],
                c_iota16.unsqueeze(1).broadcast_to((128, 10, 16)),
                jlt[:, 6:16].unsqueeze(2).broadcast_to((128, 10, 16)),
                ALU.is_equal,
            )

            # ---- path B one-hot transposes (PE; independent of the avg) ----
            ratTs = wpool.tile([64, 10, 128], BF16, tag="ratTs")
            ptgs = []
            for b0 in range(0, 10, 5):
                ptg = psT.tile([64, 5, 128], BF16, tag="ptg")
                ptgs.append(ptg)
                for m in range(b0, b0 + 5):
                    nc.tensor.transpose(ptg[:, m - b0, :], rat[:, m, :],
                                        c_identB128)

            # ---- averages (v-major psum: psv[64, v, jl]) ----
            rc = wpool.tile([64, 16], F32, tag="rc")
            nc.vector.reciprocal(rc[:], ps[:, 8, :])
            avg2 = wpool.tile([64, 16, 8], BF16, tag="avg2")
            nc.vector.tensor_tensor(
                avg2[:],
                ps[:, 0:8, :].transpose([0, 2, 1]),
                rc[:].unsqueeze(2).broadcast_to((64, 16, 8)),
                ALU.mult,
            )
            # psum->SBUF copies split across DVE and the scalar engine
            for b0 in range(0, 10, 5):
                for m in range(b0, b0 + 5):
                    eng = nc.vector if m % 2 == 0 else nc.scalar
                    if eng is nc.vector:
                        nc.vector.tensor_copy(ratTs[:, m, :],
                                              ptgs[b0 // 5][:, m - b0, :])
                    else:
                        nc.scalar.copy(ratTs[:, m, :],
                                       ptgs[b0 // 5][:, m - b0, :])

            # ---- table build: transpose + 16 K=8 replication matmuls ----
            pt = psS.tile([128, 64], BF16, tag="pt")
            nc.tensor.transpose(pt[:], avg2[:].rearrange("p a b -> p (a b)"),
                                c_identB)
            avgTs = wpool.tile([128, 64], BF16, tag="avgTs")
            nc.scalar.copy(avgTs[:], pt[:])
            tbl = psB.tile([128, 1024], F32, tag="tbl")
            for l in range(16):
                g, j8 = l // 8, l % 8
                nc.tensor.matmul(
                    tbl[:, 64 * l:64 * l + 64],
                    cb[64 * g:64 * g + 64, 736 + 128 * j8:736 + 128 * j8 + 128],
                    avgTs[64 * g:64 * g + 64, :],
                    start=True, stop=True,
                )
            tblS = wpool.tile([128, 1024], F32, tag="tblS")
            nc.scalar.copy(tblS[:], tbl[:])

            # ---- gather path B (DVE/PE): slots 6:16 via one-hot matmuls ----
            # rvp reuses the (long-free) pooling psum banks, not the table's,
            # so these matmuls don't wait for the tblS copy
            rvp = psP.tile([128, 8, 128], F32, tag="pp")
            rvp2 = psR.tile([128, 2, 128], F32, tag="tr")
            for m in range(10):
                dst = rvp[:, m, :] if m < 8 else rvp2[:, m - 8, :]
                nc.tensor.matmul(dst, ratTs[:, m, :],
                                 avg2[:].rearrange("p a b -> p (a b)"),
                                 start=True, stop=True)

            # ---- gather path A (GPSIMD): targets i=16s+q, s<6 of each group ----
            gout = wpool.tile([128, 96], F32, tag="gout")
            nc.gpsimd.ap_gather(
                gout[:].rearrange("p (n d) -> p n d", d=1),
                tblS[:].rearrange("p (n d) -> p n d", d=1),
                idx16[:],
                channels=128, num_elems=1024, d=1, num_idxs=96,
            )
            nc.sync.dma_start(outA_d[:], gout[:])

            # ---- path B select + output ----
            tmp = wpool.tile([128, 8, 8, 16], F32, tag="tmp")
            rvv = rvp[:].rearrange("p m (jl y) -> p m jl y", jl=16)
            nc.vector.tensor_tensor(
                tmp[:],
                rvv[:].transpose([0, 1, 3, 2]),
                zttl2[:, 0:8, :].unsqueeze(2).broadcast_to((128, 8, 8, 16)),
                ALU.mult,
            )
            tmp2 = wpool.tile([128, 2, 8, 16], F32, tag="tmp2")
            rvv2 = rvp2[:].rearrange("p m (jl y) -> p m jl y", jl=16)
            nc.vector.tensor_tensor(
                tmp2[:],
                rvv2[:].transpose([0, 1, 3, 2]),
                zttl2[:, 8:10, :].unsqueeze(2).broadcast_to((128, 2, 8, 16)),
                ALU.mult,
            )
            outsbB = wpool.tile([128, 10, 8], F32, tag="outsbB")
            nc.vector.tensor_reduce(outsbB[:, 0:8, :], tmp[:],
                                    axis=mybir.AxisListType.X, op=ALU.add)
            nc.vector.tensor_reduce(outsbB[:, 8:10, :], tmp2[:],
                                    axis=mybir.AxisListType.X, op=ALU.add)
            nc.scalar.dma_start(outB_d[:], outsbB[:])
    nc.compile()
    return nc


def _consts():
    cf = np.zeros((128, _CF_COLS), np.float32)
    cf[:, 0:64] = np.arange(64, dtype=np.float32)[None, :]
    cf[:, 64:80] = np.arange(16, dtype=np.float32)[None, :]

    cbv = np.zeros((128, _CB_COLS), np.float32)
    # early block: pmat | blps | identb64
    for h in range(128):
        cbv[h, h // 4] = 1.0 / 16.0
    s = 8 * np.arange(128)[:, None] + np.arange(8)[None, :]  # cell s = 8p + m
    si, sj = s // 32, s % 32
    ihj_ps = (2 * si + sj // 16)          # [128, 8]
    jl_ps = (sj % 16)
    raps = np.zeros((128, 8, 64), np.float32)
    blps = np.zeros((128, 8, 16), np.float32)
    pidx = np.arange(128)[:, None].repeat(8, 1)
    midx = np.arange(8)[None, :].repeat(128, 0)
    raps[pidx, midx, ihj_ps] = 1.0
    blps[pidx, midx, jl_ps] = 1.0
    cbv[:, 32:160] = blps.reshape(128, 128)
    cbv[0:64, 160:224] = np.eye(64, dtype=np.float32)
    # late block: raps | P64all | identb128
    cbv[:, 224:736] = raps.reshape(128, 512)
    # P64all[q, 128*j8 + (8k+y')] = ((q%64)//8 == j8) * (q%8 == y')
    q = np.arange(128)[:, None]
    mm = np.arange(128)[None, :]
    base = (q % 8 == mm % 8).astype(np.float32)  # [128, 128]
    for j8 in range(8):
        blk = base * (((q % 64) // 8) == j8)
        cbv[:, 736 + 128 * j8:736 + 128 * (j8 + 1)] = blk
    cbv[:, 1760:1888] = np.eye(128, dtype=np.float32)
    return {
        "constF": cf,
        "constB": cbv.astype(ml_dtypes.bfloat16),
    }


def _stage_core(xc_off, yc_off, yc_on, xt, b, half):
    m = {}
    fin = np.empty((128, _IN_COLS), np.float32)
    o = 0
    fin[:, o:o + KT] = xc_off[b, :, 0].reshape(KT, 128).T; o += KT
    fin[:, o:o + KT] = xc_off[b, :, 1].reshape(KT, 128).T; o += KT
    fin[:, o:o + KT * Y] = yc_off[b].reshape(KT, 128, Y).transpose(1, 0, 2) \
        .reshape(128, KT * Y); o += KT * Y
    sl = slice(half * TH, (half + 1) * TH)
    xs = xt[b, sl].reshape(8, 16, 16, 2).transpose(0, 2, 1, 3).reshape(128, NT, 2)
    fin[:, o:o + NT] = xs[:, :, 0]; o += NT
    fin[:, o:o + NT] = xs[:, :, 1]; o += NT
    m["inF"] = fin
    m["ycON"] = np.ascontiguousarray(yc_on[b].reshape(128, 1024)).astype(
        ml_dtypes.bfloat16)
    return m


_NC = None


def kernel(xc_off_grid, yc_off_grid, xc_on_grid, yc_on_grid, xt):
    global _NC
    if _NC is None:
        _NC = build_nc()
    nc = _NC
    consts = _consts()

    xc_off_grid = np.ascontiguousarray(xc_off_grid, np.float32)
    yc_off_grid = np.ascontiguousarray(yc_off_grid, np.float32)
    yc_on_grid = np.ascontiguousarray(yc_on_grid, np.float32)
    xt = np.ascontiguousarray(xt, np.float32)

    in_maps = []
    for core in range(8):
        b, half = core // 2, core % 2
        m = dict(consts)
        m.update(_stage_core(xc_off_grid, yc_off_grid, yc_on_grid, xt, b, half))
        in_maps.append(m)

    res = run_bass_kernel_spmd(nc, in_maps, list(range(8)))
    out = np.empty((B, T, Y), np.float32)
    for core in range(8):
        b, half = core // 2, core % 2
        off = half * TH
        oA = res.results[core]["outA"]          # [128=(16g+w), 96=i]
        oB = res.results[core]["outB"]          # [128=(16g+q), 10=j-6, 8=y]
        # path A: rows 256g + i (i<96), y from partition 16g+y
        a = oA.reshape(8, 16, 96)[:, 0:8, :]    # [g, y, i]
        hv = out[b]
        for g in range(8):
            hv[off + 256 * g:off + 256 * g + 96] = a[g].T
        # path B: rows 256g + 16j + q for j in [6,16)
        bv = oB.reshape(8, 16, 10, 8)           # [g, q, j-6, y]
        for g in range(8):
            blk = bv[g].transpose(1, 0, 2).reshape(160, 8)  # [(j-6, q), y]
            hv[off + 256 * g + 96:off + 256 * g + 256] = blk
    return out
